# revision 2
# baseline (speedup 1.0000x reference)
"""ConnectivityLoss Trainium2 Bass kernel.

Problem (hardcoded): pred/target (32, 1, 512, 512) f32.
  5 iterations of soft-skeletonize (3x3 min-pool -> 3x3 max-pool ->
  x = x - (M - m); both reference relus are provably no-ops), then 3x3
  sum-pool, endpoint/crossing masks, and a weighted MSE of the three
  pairs.

Sharding: pure data parallel over the batch dim; core i processes image
pairs 4i..4i+3 and returns per-partition partial sums of squared diffs;
the host sums and normalizes.

Per-core layout: partition p (128) owns image rows 4p..4p+3.
Free dims: (side 2, rowslot 4, col 512), fully contiguous.

Everything on-device runs in bf16 (numpy-validated loss error ~5.7e-3
vs the 2e-2 gate): morphology, the 3x3 sum-pool, masks and the squared
diffs.  bf16 keeps every DVE tensor_tensor in the 2x perf mode (f32 tt
runs 1x).  MSE squares+row-sums run on the Scalar engine (ACT Square
with accum_out), and the per-iteration update is fused to a single DVE
subtract by computing c = x + m on the otherwise idle GpSimd engine
during the max-pool phase (x' = x - (M - m) = c - M).

Cross-partition row shifts (rows 4p-1 / 4p+4) run on the idle
TensorEngine as shifted-identity bf16 matmuls into PSUM; ScalarE
evacuates PSUM to bf16 SBUF rows.  The hpool combine is split into
slot3 / slot0 / slots1:2 ops so the shift matmuls of the following
vpool start ~2us earlier, hiding the matmul+evac latency that
otherwise stalls the DVE ~750ns per vpool.  Compute-engine partition
windows must start at 0/32/64/96, so image boundaries are handled with
a [0:1] clipped-window overwrite (top) and matmul/consumer restricted
to [0:127) plus a [96:128) copy (bottom).
"""
import numpy as np
import ml_dtypes

import concourse.bass as bass
import concourse.tile as tile
from concourse import mybir
from concourse.bass_utils import run_bass_kernel_spmd

F32 = mybir.dt.float32
BF16 = mybir.dt.bfloat16
OP = mybir.AluOpType
AF = mybir.ActivationFunctionType

BIG = 1.0e30
P = 128
NCORES = 8
CHUNKS = 4
H = W = 512
ITERS = 5
GPFUSE = True  # compute x+m on GpSimd, fuse update to one DVE subtract

_cache = {}


def _split_waits(nc, limit=1):
    """This walrus build rejects instructions with more than ~1 embedded
    sync wait; hoist waits into standalone EventSemaphore instructions."""
    counter = 0
    for fn in nc.m.functions:
        for bb in fn.blocks:
            lst = list(bb.instructions)
            out = []
            changed = False
            for ins in lst:
                si = ins.sync_info
                waits = list(si.on_wait) if si is not None else []
                if len(waits) > limit:
                    changed = True
                    for w in waits:
                        counter += 1
                        es = mybir.InstEventSemaphore(
                            name=f"I-wsplit-{counter}", ins=[], outs=[],
                            sync_info=mybir.SyncInfo(on_wait=[w], on_update=[]),
                            bass_nofuse=True,
                        )
                        es.engine = ins.engine
                        out.append(es)
                    ins.sync_info = mybir.SyncInfo(
                        on_wait=[], on_update=list(si.on_update))
                out.append(ins)
            if changed:
                bb.instructions = out
    return counter


def _shift_mats():
    sup = np.zeros((P, P), np.float32)   # psum[p] = rhs[p-1]; col 0 zero
    sdn = np.zeros((P, P), np.float32)   # psum[p] = rhs[p+1]; use cols 0:127
    for p in range(1, P):
        sup[p - 1, p] = 1.0
    for p in range(P - 1):
        sdn[p + 1, p] = 1.0
    return (sup.astype(ml_dtypes.bfloat16), sdn.astype(ml_dtypes.bfloat16))


def _build():
    nc = bass.Bass()
    pred = nc.dram_tensor("pred", [CHUNKS, H, W], F32, kind="ExternalInput")
    targ = nc.dram_tensor("targ", [CHUNKS, H, W], F32, kind="ExternalInput")
    supd = nc.dram_tensor("sup", [P, P], BF16, kind="ExternalInput")
    sdnd = nc.dram_tensor("sdn", [P, P], BF16, kind="ExternalInput")
    parts = nc.dram_tensor("partials", [P, CHUNKS * 3], F32,
                           kind="ExternalOutput")
    pred_v = pred.rearrange("n (p s) c -> n p s c", s=4)
    targ_v = targ.rearrange("n (p s) c -> n p s c", s=4)

    with tile.TileContext(nc) as tc:
        with tc.tile_pool(name="bufs", bufs=1) as pool, \
             tc.tile_pool(name="ps", bufs=1, space="PSUM") as pp:
            sh4 = [P, 2, 4, W]
            # bf16 morphology buffers
            xa = pool.tile(sh4, BF16)
            xb = pool.tile(sh4, BF16)
            t = pool.tile(sh4, BF16)
            m = pool.tile(sh4, BF16)
            Mh = pool.tile(sh4, BF16)
            sk = pool.tile(sh4, BF16)
            t5 = pool.tile([P, 2, 5, W], BF16)
            tmin = pool.tile([P, 2, 4, W + 1], BF16)   # +BIG pad cols 0,512
            tmax = pool.tile([P, 2, 4, W + 1], BF16)   # -BIG pad cols 0,512
            # load + bf16 post buffers
            stage = pool.tile(sh4, F32)
            scr = pool.tile(sh4, BF16)
            shb = pool.tile(sh4, BF16)
            ncb = pool.tile(sh4, BF16)
            onb = pool.tile(sh4, BF16)
            epb = pool.tile(sh4, BF16)
            crb = pool.tile(sh4, BF16)
            sup = pool.tile([P, P], BF16)
            sdn = pool.tile([P, P], BF16)
            pt = pool.tile([P, CHUNKS * 3], F32)
            pu = pp.tile([P, 2, W], F32)
            pd = pp.tile([P, 2, W], F32)

            nc.sync.dma_start(out=sup, in_=supd[:])
            nc.sync.dma_start(out=sdn, in_=sdnd[:])
            # only the pad columns (0 and 512) need the +/-BIG sentinel
            nc.vector.memset(tmin[:, :, :, 0:1], BIG)
            nc.vector.memset(tmin[:, :, :, W:W + 1], BIG)
            nc.vector.memset(tmax[:, :, :, 0:1], -BIG)
            nc.vector.memset(tmax[:, :, :, W:W + 1], -BIG)

            def tt(out, a, b, op):
                nc.vector.tensor_tensor(out=out, in0=a, in1=b, op=op)

            def hpool(dst, src, op, split=True):
                # dst = 3-wide col pool of src (SAME, clipped). The pair
                # temp has static +/-BIG pad cols, so the second op covers
                # the edge columns too.  The combine is split so slots 3/0
                # land first: they feed the next vpool's shift matmuls.
                tp = tmin if op == OP.min else tmax
                tt(tp[:, :, :, 1:512], src[:, :, :, 0:511],
                   src[:, :, :, 1:512], op)
                if split:
                    tt(dst[:, :, 3, 0:512], tp[:, :, 3, 0:512],
                       tp[:, :, 3, 1:513], op)
                    tt(dst[:, :, 0, 0:512], tp[:, :, 0, 0:512],
                       tp[:, :, 0, 1:513], op)
                    tt(dst[:, :, 1:3, 0:512], tp[:, :, 1:3, 0:512],
                       tp[:, :, 1:3, 1:513], op)
                else:
                    tt(dst[:, :, :, 0:512], tp[:, :, :, 0:512],
                       tp[:, :, :, 1:513], op)

            def vshift(src):
                # t5 slot0[p] = src[p-1, slot3], t5 slot4[p] = src[p+1, slot0]
                # (slot0 row 0 is matmul-zero garbage; the consumer's row 0
                # is overwritten with the clipped-window value instead)
                nc.tensor.matmul(pu[:, 0], sup[:], src[:, 0, 3, :])
                nc.tensor.matmul(pu[:, 1], sup[:], src[:, 1, 3, :])
                nc.scalar.copy(out=t5[:, :, 0, :], in_=pu)    # f32 -> bf16
                nc.tensor.matmul(pd[0:127, 0], sdn[:, 0:127],
                                 src[:, 0, 0, :])
                nc.tensor.matmul(pd[0:127, 1], sdn[:, 0:127],
                                 src[:, 1, 0, :])
                nc.scalar.copy(out=t5[0:127, :, 4, :], in_=pd[0:127])

            def vpool(dst, src, op):
                # dst = 3-wide row pool of src across partitions;
                # t5 = [shift-up, pair01, pair12, pair23, shift-dn]
                vshift(src)
                tt(t5[:, :, 1:4, :], src[:, :, 0:3, :],
                   src[:, :, 1:4, :], op)
                tt(dst[:, :, 0:3, :], t5[:, :, 0:3, :],
                   t5[:, :, 1:4, :], op)
                # image-boundary rows: clipped windows (partition starts must
                # be quadrant-aligned, so write [0:1] / [96:128] on ACT)
                nc.scalar.copy(out=dst[0:1, :, 0, :],
                               in_=t5[0:1, :, 1, :])
                nc.scalar.copy(out=dst[96:128, :, 3, :],
                               in_=t5[96:128, :, 3, :])
                tt(dst[0:127, :, 3, :], t5[0:127, :, 3, :],
                   t5[0:127, :, 4, :], op)

            for ch in range(CHUNKS):
                x, other = (xa, xb) if ch % 2 == 0 else (xb, xa)
                nc.sync.dma_start(out=stage[:, 0], in_=pred_v[ch])
                nc.gpsimd.dma_start(out=stage[:, 1], in_=targ_v[ch])
                nc.scalar.copy(out=x[:, 0], in_=stage[:, 0])  # f32 -> bf16
                nc.scalar.copy(out=x[:, 1], in_=stage[:, 1])

                for it in range(ITERS):
                    mh = other
                    hpool(mh, x, OP.min)      # mh = minc3(x)
                    vpool(m, mh, OP.min)      # m = minpool3(x)
                    if GPFUSE:
                        # c = x + m on GpSimd, overlapped with max-pool
                        nc.gpsimd.tensor_tensor(out=t, in0=x, in1=m,
                                                op=OP.add)
                    hpool(mh, m, OP.max)      # mh reused for maxc3(m)
                    vpool(Mh, mh, OP.max)     # Mh = M = maxpool3(m)
                    out_x = sk if it == ITERS - 1 else mh
                    if GPFUSE:
                        tt(out_x[:, :, :, :], t[:, :, :, :], Mh[:, :, :, :],
                           OP.subtract)       # x' = (x+m) - M
                    else:
                        tt(t[:, :, :, :], Mh[:, :, :, :], m[:, :, :, :],
                           OP.subtract)       # contour
                        tt(out_x[:, :, :, :], x[:, :, :, :], t[:, :, :, :],
                           OP.subtract)
                    if it < ITERS - 1:
                        x, other = mh, x

                # ncnt = 3x3 sum-pool of sk, all bf16
                tt(scr[:, :, :, 0:511], sk[:, :, :, 0:511],
                   sk[:, :, :, 1:512], OP.add)
                tt(shb[:, :, 3, 1:511], scr[:, :, 3, 0:510],
                   sk[:, :, 3, 2:512], OP.add)
                tt(shb[:, :, 0, 1:511], scr[:, :, 0, 0:510],
                   sk[:, :, 0, 2:512], OP.add)
                tt(shb[:, :, 1:3, 1:511], scr[:, :, 1:3, 0:510],
                   sk[:, :, 1:3, 2:512], OP.add)
                nc.scalar.copy(out=shb[:, :, :, 0:1], in_=scr[:, :, :, 0:1])
                nc.scalar.copy(out=shb[:, :, :, 511:512],
                               in_=scr[:, :, :, 510:511])
                # vertical sum via slot pairs + cross-partition shift rows
                vshift(shb)
                tt(scr[:, :, 1:4, :], shb[:, :, 0:3, :], shb[:, :, 1:4, :],
                   OP.add)
                tt(ncb[:, :, 1:3, :], scr[:, :, 1:3, :], shb[:, :, 2:4, :],
                   OP.add)
                tt(ncb[:, :, 0, :], scr[:, :, 1, :], t5[:, :, 0, :], OP.add)
                nc.scalar.copy(out=ncb[96:128, :, 3, :],
                               in_=scr[96:128, :, 3, :])
                tt(ncb[0:127, :, 3, :], scr[0:127, :, 3, :],
                   t5[0:127, :, 4, :], OP.add)
                # on = sk > 0.5 ; ep = (ncnt == 2)*on ; cr = (ncnt >= 4)*on
                nc.vector.tensor_scalar(out=onb[:, :, :, :],
                                        in0=sk[:, :, :, :],
                                        scalar1=0.5, scalar2=None,
                                        op0=OP.is_gt)
                for side in range(2):  # stt requires <=3D APs
                    nc.vector.scalar_tensor_tensor(
                        out=epb[:, side], in0=ncb[:, side], scalar=2.0,
                        in1=onb[:, side], op0=OP.is_equal, op1=OP.mult)
                    nc.vector.scalar_tensor_tensor(
                        out=crb[:, side], in0=ncb[:, side], scalar=4.0,
                        in1=onb[:, side], op0=OP.is_ge, op1=OP.mult)
                # squared-diff partial sums: diff on DVE (bf16 2x),
                # square + row-sum on ScalarE (Square + accum_out, f32)
                for k, buf in enumerate((sk, epb, crb)):
                    tt(scr[:, 0], buf[:, 0], buf[:, 1], OP.subtract)
                    nc.scalar.activation(
                        out=scr[:, 1], in_=scr[:, 0], func=AF.Square,
                        accum_out=pt[:, ch * 3 + k: ch * 3 + k + 1])

            nc.sync.dma_start(out=parts[:], in_=pt)

    _split_waits(nc, limit=1)
    return nc


def _run(pred_np, targ_np, trace=False):
    if "nc" not in _cache:
        _cache["nc"] = _build()
    nc = _cache["nc"]
    sup, sdn = _shift_mats()
    in_maps = []
    for c in range(NCORES):
        in_maps.append({
            "pred": np.ascontiguousarray(pred_np[c * CHUNKS:(c + 1) * CHUNKS]),
            "targ": np.ascontiguousarray(targ_np[c * CHUNKS:(c + 1) * CHUNKS]),
            "sup": sup, "sdn": sdn,
        })
    return run_bass_kernel_spmd(nc, in_maps, core_ids=list(range(NCORES)),
                                trace=trace)


def kernel(pred, target):
    pred_np = np.asarray(pred, dtype=np.float32).reshape(32, H, W)
    targ_np = np.asarray(target, dtype=np.float32).reshape(32, H, W)
    res = _run(pred_np, targ_np)
    sums = np.zeros(3, dtype=np.float64)
    for r in res.results:
        p = r["partials"].astype(np.float64).reshape(P, CHUNKS, 3)
        sums += p.sum(axis=(0, 1))
    n = 32.0 * H * W
    loss = 0.6 * sums[0] / n + 0.2 * sums[1] / n + 0.2 * sums[2] / n
    return np.float32(loss)


# revision 7
# speedup vs baseline: 1.4258x; 1.4258x over previous
"""ConnectivityLoss Trainium2 Bass kernel.

Problem (hardcoded): pred/target (32, 1, 512, 512) f32.
  5 iterations of soft-skeletonize (3x3 min-pool -> 3x3 max-pool ->
  x = x - (M - m); both reference relus are provably no-ops), then 3x3
  sum-pool, endpoint/crossing masks, and a weighted MSE of the three
  pairs.

Sharding: pure data parallel over the batch dim; core i processes image
pairs 4i..4i+3 and returns per-partition partial sums of squared diffs;
the host sums and normalizes.

Per-core layout: partition p (128) owns image rows 4p..4p+3.
Free dims: (side 2, rowslot 4, col 512), fully contiguous.

Everything on-device runs in bf16 (numpy-validated loss error ~5.7e-3
vs the 2e-2 gate): morphology, the 3x3 sum-pool, masks and the squared
diffs.  bf16 keeps every DVE tensor_tensor in the 2x perf mode (f32 tt
runs 1x).  MSE squares+row-sums run on the Scalar engine (ACT Square
with accum_out), and the per-iteration update is fused to a single DVE
subtract by computing c = x + m on the otherwise idle GpSimd engine
during the max-pool phase (x' = x - (M - m) = c - M).

Cross-partition row shifts (rows 4p-1 / 4p+4) run on the idle
TensorEngine as shifted-identity bf16 matmuls into PSUM; ScalarE
evacuates PSUM to bf16 SBUF rows.  The hpool combine is split into
slot3 / slot0 / slots1:2 ops so the shift matmuls of the following
vpool start ~2us earlier, hiding the matmul+evac latency that
otherwise stalls the DVE ~750ns per vpool.  Compute-engine partition
windows must start at 0/32/64/96, so image boundaries are handled with
a [0:1] clipped-window overwrite (top) and matmul/consumer restricted
to [0:127) plus a [96:128) copy (bottom).
"""
import numpy as np
import ml_dtypes

import concourse.bass as bass
import concourse.tile as tile
from concourse import mybir
from concourse.bass_utils import run_bass_kernel_spmd

F32 = mybir.dt.float32
BF16 = mybir.dt.bfloat16
OP = mybir.AluOpType
AF = mybir.ActivationFunctionType

BIG = 1.0e30
P = 128
NCORES = 8
CHUNKS = 4
H = W = 512
ITERS = 5
GPFUSE = False  # GpSimd tt shares the DVE SBUF port: measured -48% on DVE

_cache = {}


def _split_waits(nc, limit=1):
    """This walrus build rejects instructions with more than ~1 embedded
    sync wait; hoist waits into standalone EventSemaphore instructions."""
    counter = 0
    for fn in nc.m.functions:
        for bb in fn.blocks:
            lst = list(bb.instructions)
            out = []
            changed = False
            for ins in lst:
                si = ins.sync_info
                waits = list(si.on_wait) if si is not None else []
                if len(waits) > limit:
                    changed = True
                    for w in waits:
                        counter += 1
                        es = mybir.InstEventSemaphore(
                            name=f"I-wsplit-{counter}", ins=[], outs=[],
                            sync_info=mybir.SyncInfo(on_wait=[w], on_update=[]),
                            bass_nofuse=True,
                        )
                        es.engine = ins.engine
                        out.append(es)
                    ins.sync_info = mybir.SyncInfo(
                        on_wait=[], on_update=list(si.on_update))
                out.append(ins)
            if changed:
                bb.instructions = out
    return counter


def _shift_mats():
    sup = np.zeros((P, P), np.float32)   # psum[p] = rhs[p-1]; col 0 zero
    sdn = np.zeros((P, P), np.float32)   # psum[p] = rhs[p+1]; use cols 0:127
    for p in range(1, P):
        sup[p - 1, p] = 1.0
    for p in range(P - 1):
        sdn[p + 1, p] = 1.0
    return (sup.astype(ml_dtypes.bfloat16), sdn.astype(ml_dtypes.bfloat16))


def _build():
    nc = bass.Bass()
    pred = nc.dram_tensor("pred", [CHUNKS, H, W], F32, kind="ExternalInput")
    targ = nc.dram_tensor("targ", [CHUNKS, H, W], F32, kind="ExternalInput")
    supd = nc.dram_tensor("sup", [P, P], BF16, kind="ExternalInput")
    sdnd = nc.dram_tensor("sdn", [P, P], BF16, kind="ExternalInput")
    parts = nc.dram_tensor("partials", [P, CHUNKS * 3], F32,
                           kind="ExternalOutput")
    pred_v = pred.rearrange("n (p s) c -> n p s c", s=4)
    targ_v = targ.rearrange("n (p s) c -> n p s c", s=4)

    with tile.TileContext(nc) as tc:
        with tc.tile_pool(name="bufs", bufs=1) as pool, \
             tc.tile_pool(name="ps", bufs=1, space="PSUM") as pp:
            sh4 = [P, 2, 4, W]
            # bf16 morphology buffers
            xa = pool.tile(sh4, BF16)
            xb = pool.tile(sh4, BF16)
            t = pool.tile(sh4, BF16)
            m = pool.tile(sh4, BF16)
            Mh = pool.tile(sh4, BF16)
            sk = pool.tile(sh4, BF16)
            t5 = pool.tile([P, 2, 5, W], BF16)
            tmin = pool.tile([P, 2, 4, W + 1], BF16)   # +BIG pad cols 0,512
            tmax = pool.tile([P, 2, 4, W + 1], BF16)   # -BIG pad cols 0,512
            # load + bf16 post buffers
            stage = pool.tile(sh4, F32)
            scr = pool.tile(sh4, BF16)
            shb = pool.tile(sh4, BF16)
            ncb = pool.tile(sh4, BF16)
            onb = pool.tile(sh4, BF16)
            epb = pool.tile(sh4, BF16)
            crb = pool.tile(sh4, BF16)
            sup = pool.tile([P, P], BF16)
            sdn = pool.tile([P, P], BF16)
            pt = pool.tile([P, CHUNKS * 3], F32)
            # shift-row landing zones for the post-pool phase (decoupled
            # from t5/pu/pd so the sum-pool overlaps the next chunk's
            # morphology)
            qu = pool.tile([P, 2, W], BF16)
            qd = pool.tile([P, 2, W], BF16)
            pu = pp.tile([P, 2, W], F32)
            pd = pp.tile([P, 2, W], F32)
            pu2 = pp.tile([P, 2, W], F32)
            pd2 = pp.tile([P, 2, W], F32)

            nc.sync.dma_start(out=sup, in_=supd[:])
            nc.sync.dma_start(out=sdn, in_=sdnd[:])
            # only the pad columns (0 and 512) need the +/-BIG sentinel
            nc.vector.memset(tmin[:, :, :, 0:1], BIG)
            nc.vector.memset(tmin[:, :, :, W:W + 1], BIG)
            nc.vector.memset(tmax[:, :, :, 0:1], -BIG)
            nc.vector.memset(tmax[:, :, :, W:W + 1], -BIG)

            def tt(out, a, b, op):
                nc.vector.tensor_tensor(out=out, in0=a, in1=b, op=op)

            def hpool(dst, src, op, split=True):
                # dst = 3-wide col pool of src (SAME, clipped). The pair
                # temp has static +/-BIG pad cols, so the second op covers
                # the edge columns too.  The combine is split so slots 3/0
                # land first: they feed the next vpool's shift matmuls.
                tp = tmin if op == OP.min else tmax
                tt(tp[:, :, :, 1:512], src[:, :, :, 0:511],
                   src[:, :, :, 1:512], op)
                if split:
                    tt(dst[:, :, 3, 0:512], tp[:, :, 3, 0:512],
                       tp[:, :, 3, 1:513], op)
                    tt(dst[:, :, 0, 0:512], tp[:, :, 0, 0:512],
                       tp[:, :, 0, 1:513], op)
                    tt(dst[:, :, 1:3, 0:512], tp[:, :, 1:3, 0:512],
                       tp[:, :, 1:3, 1:513], op)
                else:
                    tt(dst[:, :, :, 0:512], tp[:, :, :, 0:512],
                       tp[:, :, :, 1:513], op)

            def vshift(src):
                # t5 slot0[p] = src[p-1, slot3], t5 slot4[p] = src[p+1, slot0]
                # (slot0 row 0 is matmul-zero garbage; the consumer's row 0
                # is overwritten with the clipped-window value instead)
                nc.tensor.matmul(pu[:, 0], sup[:], src[:, 0, 3, :])
                nc.tensor.matmul(pu[:, 1], sup[:], src[:, 1, 3, :])
                nc.scalar.copy(out=t5[:, :, 0, :], in_=pu)    # f32 -> bf16
                nc.tensor.matmul(pd[0:127, 0], sdn[:, 0:127],
                                 src[:, 0, 0, :])
                nc.tensor.matmul(pd[0:127, 1], sdn[:, 0:127],
                                 src[:, 1, 0, :])
                nc.scalar.copy(out=t5[0:127, :, 4, :], in_=pd[0:127])

            def vpool(dst, src, op):
                # dst = 3-wide row pool of src across partitions;
                # t5 = [shift-up, pair01, pair12, pair23, shift-dn]
                vshift(src)
                tt(t5[:, :, 1:4, :], src[:, :, 0:3, :],
                   src[:, :, 1:4, :], op)
                tt(dst[:, :, 0:3, :], t5[:, :, 0:3, :],
                   t5[:, :, 1:4, :], op)
                # image-boundary rows: clipped windows (partition starts must
                # be quadrant-aligned, so write [0:1] / [96:128] on ACT)
                nc.scalar.copy(out=dst[0:1, :, 0, :],
                               in_=t5[0:1, :, 1, :])
                nc.scalar.copy(out=dst[96:128, :, 3, :],
                               in_=t5[96:128, :, 3, :])
                tt(dst[0:127, :, 3, :], t5[0:127, :, 3, :],
                   t5[0:127, :, 4, :], op)

            for ch in range(CHUNKS):
                x, other = (xa, xb) if ch % 2 == 0 else (xb, xa)
                # spread the load over the three available DMA queues
                nc.sync.dma_start(out=stage[:, 0, 0:2], in_=pred_v[ch, :, 0:2])
                nc.scalar.dma_start(out=stage[:, 0, 2:4],
                                    in_=pred_v[ch, :, 2:4])
                nc.gpsimd.dma_start(out=stage[:, 1, 0:2],
                                    in_=targ_v[ch, :, 0:2])
                nc.sync.dma_start(out=stage[:, 1, 2:4],
                                  in_=targ_v[ch, :, 2:4])
                nc.scalar.copy(out=x[:, 0], in_=stage[:, 0])  # f32 -> bf16
                nc.scalar.copy(out=x[:, 1], in_=stage[:, 1])

                for it in range(ITERS):
                    mh = other
                    hpool(mh, x, OP.min)      # mh = minc3(x)
                    vpool(m, mh, OP.min)      # m = minpool3(x)
                    if GPFUSE:
                        # c = x + m on GpSimd, overlapped with max-pool
                        nc.gpsimd.tensor_tensor(out=t, in0=x, in1=m,
                                                op=OP.add)
                    hpool(mh, m, OP.max)      # mh reused for maxc3(m)
                    vpool(Mh, mh, OP.max)     # Mh = M = maxpool3(m)
                    out_x = sk if it == ITERS - 1 else mh
                    if GPFUSE:
                        tt(out_x[:, :, :, :], t[:, :, :, :], Mh[:, :, :, :],
                           OP.subtract)       # x' = (x+m) - M
                    else:
                        tt(t[:, :, :, :], Mh[:, :, :, :], m[:, :, :, :],
                           OP.subtract)       # contour
                        tt(out_x[:, :, :, :], x[:, :, :, :], t[:, :, :, :],
                           OP.subtract)
                    if it < ITERS - 1:
                        x, other = mh, x

                # ncnt = 3x3 sum-pool of sk, all bf16
                tt(scr[:, :, :, 0:511], sk[:, :, :, 0:511],
                   sk[:, :, :, 1:512], OP.add)
                tt(shb[:, :, 3, 1:511], scr[:, :, 3, 0:510],
                   sk[:, :, 3, 2:512], OP.add)
                tt(shb[:, :, 0, 1:511], scr[:, :, 0, 0:510],
                   sk[:, :, 0, 2:512], OP.add)
                tt(shb[:, :, 1:3, 1:511], scr[:, :, 1:3, 0:510],
                   sk[:, :, 1:3, 2:512], OP.add)
                nc.scalar.copy(out=shb[:, :, :, 0:1], in_=scr[:, :, :, 0:1])
                nc.scalar.copy(out=shb[:, :, :, 511:512],
                               in_=scr[:, :, :, 510:511])
                # vertical sum via slot pairs + cross-partition shift rows
                # (dedicated qu/qd/pu2/pd2 so this phase never blocks the
                # next chunk's morphology shift pipeline)
                nc.tensor.matmul(pu2[:, 0], sup[:], shb[:, 0, 3, :])
                nc.tensor.matmul(pu2[:, 1], sup[:], shb[:, 1, 3, :])
                nc.scalar.copy(out=qu, in_=pu2)               # f32 -> bf16
                nc.tensor.matmul(pd2[0:127, 0], sdn[:, 0:127],
                                 shb[:, 0, 0, :])
                nc.tensor.matmul(pd2[0:127, 1], sdn[:, 0:127],
                                 shb[:, 1, 0, :])
                nc.scalar.copy(out=qd[0:127], in_=pd2[0:127])
                tt(scr[:, :, 1:4, :], shb[:, :, 0:3, :], shb[:, :, 1:4, :],
                   OP.add)
                tt(ncb[:, :, 1:3, :], scr[:, :, 1:3, :], shb[:, :, 2:4, :],
                   OP.add)
                tt(ncb[:, :, 0, :], scr[:, :, 1, :], qu[:], OP.add)
                nc.scalar.copy(out=ncb[96:128, :, 3, :],
                               in_=scr[96:128, :, 3, :])
                tt(ncb[0:127, :, 3, :], scr[0:127, :, 3, :],
                   qd[0:127], OP.add)
                # on = sk > 0.5 ; ep = (ncnt == 2)*on ; cr = (ncnt >= 4)*on
                # (tensor_scalar runs 4x on bf16; stt has no 2x uop)
                nc.vector.tensor_scalar(out=onb[:, :, :, :],
                                        in0=sk[:, :, :, :],
                                        scalar1=0.5, scalar2=None,
                                        op0=OP.is_gt)
                nc.vector.tensor_scalar(out=shb[:, :, :, :],
                                        in0=ncb[:, :, :, :],
                                        scalar1=2.0, scalar2=None,
                                        op0=OP.is_equal)
                tt(epb[:, :, :, :], shb[:, :, :, :], onb[:, :, :, :],
                   OP.mult)
                nc.vector.tensor_scalar(out=shb[:, :, :, :],
                                        in0=ncb[:, :, :, :],
                                        scalar1=4.0, scalar2=None,
                                        op0=OP.is_ge)
                tt(crb[:, :, :, :], shb[:, :, :, :], onb[:, :, :, :],
                   OP.mult)
                # squared-diff partial sums: diff on DVE (bf16 2x),
                # square + row-sum on ScalarE (Square + accum_out, f32)
                for k, buf in enumerate((sk, epb, crb)):
                    tt(scr[:, 0], buf[:, 0], buf[:, 1], OP.subtract)
                    nc.scalar.activation(
                        out=scr[:, 1], in_=scr[:, 0], func=AF.Square,
                        accum_out=pt[:, ch * 3 + k: ch * 3 + k + 1])

            nc.sync.dma_start(out=parts[:], in_=pt)

    _split_waits(nc, limit=1)
    return nc


def _run(pred_np, targ_np, trace=False):
    if "nc" not in _cache:
        _cache["nc"] = _build()
    nc = _cache["nc"]
    sup, sdn = _shift_mats()
    in_maps = []
    for c in range(NCORES):
        in_maps.append({
            "pred": np.ascontiguousarray(pred_np[c * CHUNKS:(c + 1) * CHUNKS]),
            "targ": np.ascontiguousarray(targ_np[c * CHUNKS:(c + 1) * CHUNKS]),
            "sup": sup, "sdn": sdn,
        })
    return run_bass_kernel_spmd(nc, in_maps, core_ids=list(range(NCORES)),
                                trace=trace)


def kernel(pred, target):
    pred_np = np.asarray(pred, dtype=np.float32).reshape(32, H, W)
    targ_np = np.asarray(target, dtype=np.float32).reshape(32, H, W)
    res = _run(pred_np, targ_np)
    sums = np.zeros(3, dtype=np.float64)
    for r in res.results:
        p = r["partials"].astype(np.float64).reshape(P, CHUNKS, 3)
        sums += p.sum(axis=(0, 1))
    n = 32.0 * H * W
    loss = 0.6 * sums[0] / n + 0.2 * sums[1] / n + 0.2 * sums[2] / n
    return np.float32(loss)


# revision 17
# speedup vs baseline: 1.5170x; 1.0640x over previous
"""ConnectivityLoss Trainium2 Bass kernel.

Problem (hardcoded): pred/target (32, 1, 512, 512) f32.
  5 iterations of soft-skeletonize (3x3 min-pool -> 3x3 max-pool ->
  x = x - (M - m); both reference relus are provably no-ops), then 3x3
  sum-pool, endpoint/crossing masks, and a weighted MSE of the three
  pairs.

Sharding: pure data parallel over the batch dim; core i processes image
pairs 4i..4i+3 and returns per-partition partial sums of squared diffs;
the host sums and normalizes.

Per-core layout: partition p (128) owns image rows 4p..4p+3.
Free dims: (side 2, rowslot 4, col 512), fully contiguous.

Everything on-device runs in bf16 (numpy-validated loss error ~5.7e-3
vs the 2e-2 gate): morphology, the 3x3 sum-pool, masks and the squared
diffs.  bf16 keeps every DVE tensor_tensor in the 2x perf mode (f32 tt
runs 1x).  MSE squares+row-sums run on the Scalar engine (ACT Square
with accum_out), and the per-iteration update is fused to a single DVE
subtract by computing c = x + m on the otherwise idle GpSimd engine
during the max-pool phase (x' = x - (M - m) = c - M).

Cross-partition row shifts (rows 4p-1 / 4p+4) run on the idle
TensorEngine as shifted-identity bf16 matmuls into PSUM; ScalarE
evacuates PSUM to bf16 SBUF rows.  The hpool combine is split into
slot3 / slot0 / slots1:2 ops so the shift matmuls of the following
vpool start ~2us earlier, hiding the matmul+evac latency that
otherwise stalls the DVE ~750ns per vpool.  Compute-engine partition
windows must start at 0/32/64/96, so image boundaries are handled with
a [0:1] clipped-window overwrite (top) and matmul/consumer restricted
to [0:127) plus a [96:128) copy (bottom).
"""
import numpy as np
import ml_dtypes

import concourse.bass as bass
import concourse.tile as tile
from concourse import mybir
from concourse.bass_utils import run_bass_kernel_spmd

F32 = mybir.dt.float32
BF16 = mybir.dt.bfloat16
OP = mybir.AluOpType
AF = mybir.ActivationFunctionType

BIG = 1.0e30
P = 128
NCORES = 8
CHUNKS = 4
H = W = 512
ITERS = 5
GPFUSE = False  # GpSimd tt shares the DVE SBUF port: measured -48% on DVE

_cache = {}


def _split_waits(nc, limit=1):
    """This walrus build rejects instructions with more than ~1 embedded
    sync wait; hoist waits into standalone EventSemaphore instructions."""
    counter = 0
    for fn in nc.m.functions:
        for bb in fn.blocks:
            lst = list(bb.instructions)
            out = []
            changed = False
            for ins in lst:
                si = ins.sync_info
                waits = list(si.on_wait) if si is not None else []
                if len(waits) > limit:
                    changed = True
                    for w in waits:
                        counter += 1
                        es = mybir.InstEventSemaphore(
                            name=f"I-wsplit-{counter}", ins=[], outs=[],
                            sync_info=mybir.SyncInfo(on_wait=[w], on_update=[]),
                            bass_nofuse=True,
                        )
                        es.engine = ins.engine
                        out.append(es)
                    ins.sync_info = mybir.SyncInfo(
                        on_wait=[], on_update=list(si.on_update))
                out.append(ins)
            if changed:
                bb.instructions = out
    return counter


def _shift_mats():
    sup = np.zeros((P, P), np.float32)   # psum[p] = rhs[p-1]; col 0 zero
    sdn = np.zeros((P, P), np.float32)   # psum[p] = rhs[p+1]; col 127 zero
    for p in range(1, P):
        sup[p - 1, p] = 1.0
    for p in range(P - 1):
        sdn[p + 1, p] = 1.0
    bvec = np.zeros((P, 4), np.float32)
    bvec[0, 0] = BIG      # min, shift-up sentinel at row 0
    bvec[127, 1] = BIG    # min, shift-down sentinel at row 511
    bvec[0, 2] = -BIG     # max
    bvec[127, 3] = -BIG
    return (sup.astype(ml_dtypes.bfloat16), sdn.astype(ml_dtypes.bfloat16),
            bvec)


def _build():
    nc = bass.Bass()
    pred = nc.dram_tensor("pred", [CHUNKS, H, W], F32, kind="ExternalInput")
    targ = nc.dram_tensor("targ", [CHUNKS, H, W], F32, kind="ExternalInput")
    supd = nc.dram_tensor("sup", [P, P], BF16, kind="ExternalInput")
    sdnd = nc.dram_tensor("sdn", [P, P], BF16, kind="ExternalInput")
    # per-partition bias columns for the shift-row evacuations:
    # col0 +BIG@p0, col1 +BIG@p127 (min), col2 -BIG@p0, col3 -BIG@p127 (max)
    bvcd = nc.dram_tensor("bvec", [P, 4], F32, kind="ExternalInput")
    parts = nc.dram_tensor("partials", [P, CHUNKS * 3], F32,
                           kind="ExternalOutput")
    pred_v = pred.rearrange("n (p s) c -> n p s c", s=4)
    targ_v = targ.rearrange("n (p s) c -> n p s c", s=4)

    with tile.TileContext(nc) as tc:
        with tc.tile_pool(name="bufs", bufs=1) as pool, \
             tc.tile_pool(name="ps", bufs=1, space="PSUM") as pp:
            sh4 = [P, 2, 4, W]
            # bf16 morphology buffers
            xa = pool.tile(sh4, BF16)
            xb = pool.tile(sh4, BF16)
            t = pool.tile(sh4, BF16)
            m = pool.tile(sh4, BF16)
            Mh = pool.tile(sh4, BF16)
            sk = pool.tile(sh4, BF16)
            t5 = pool.tile([P, 2, 5, W], BF16)
            tmin = pool.tile([P, 2, 4, W + 1], BF16)   # +BIG pad cols 0,512
            tmax = pool.tile([P, 2, 4, W + 1], BF16)   # -BIG pad cols 0,512
            # load + bf16 post buffers
            stage = pool.tile(sh4, F32)
            scr = pool.tile(sh4, BF16)
            shb = pool.tile(sh4, BF16)
            ncb = pool.tile(sh4, BF16)
            onb = pool.tile(sh4, BF16)
            epb = pool.tile(sh4, BF16)
            crb = pool.tile(sh4, BF16)
            sup = pool.tile([P, P], BF16)
            sdn = pool.tile([P, P], BF16)
            bvec = pool.tile([P, 4], F32)
            pt = pool.tile([P, CHUNKS * 3], F32)
            # shift-row landing zones for the post-pool phase (decoupled
            # from t5/pu/pd so the sum-pool overlaps the next chunk's
            # morphology)
            qu = pool.tile([P, 2, W], BF16)
            qd = pool.tile([P, 2, W], BF16)
            pu = pp.tile([P, 2, W], F32)
            pd = pp.tile([P, 2, W], F32)
            pu2 = pp.tile([P, 2, W], F32)
            pd2 = pp.tile([P, 2, W], F32)

            nc.sync.dma_start(out=sup, in_=supd[:])
            nc.sync.dma_start(out=sdn, in_=sdnd[:])
            nc.sync.dma_start(out=bvec, in_=bvcd[:])
            # only the pad columns (0 and 512) need the +/-BIG sentinel
            nc.vector.memset(tmin[:, :, :, 0:1], BIG)
            nc.vector.memset(tmin[:, :, :, W:W + 1], BIG)
            nc.vector.memset(tmax[:, :, :, 0:1], -BIG)
            nc.vector.memset(tmax[:, :, :, W:W + 1], -BIG)

            def tt(out, a, b, op):
                nc.vector.tensor_tensor(out=out, in0=a, in1=b, op=op)

            def hpool(dst, src, op, split=True):
                # dst = 3-wide col pool of src (SAME, clipped). The pair
                # temp has static +/-BIG pad cols, so the second op covers
                # the edge columns too.  The combine is split so slots 3/0
                # land first: they feed the next vpool's shift matmuls.
                tp = tmin if op == OP.min else tmax
                tt(tp[:, :, :, 1:512], src[:, :, :, 0:511],
                   src[:, :, :, 1:512], op)
                if split:
                    tt(dst[:, :, 3, 0:512], tp[:, :, 3, 0:512],
                       tp[:, :, 3, 1:513], op)
                    tt(dst[:, :, 0, 0:512], tp[:, :, 0, 0:512],
                       tp[:, :, 0, 1:513], op)
                    tt(dst[:, :, 1:3, 0:512], tp[:, :, 1:3, 0:512],
                       tp[:, :, 1:3, 1:513], op)
                else:
                    tt(dst[:, :, :, 0:512], tp[:, :, :, 0:512],
                       tp[:, :, :, 1:513], op)

            def vpool(dst, src, op):
                # dst = 3-wide row pool of src across partitions;
                # t5 = [shift-up, pair01, pair12, pair23, shift-dn].
                # Shift rows land via ACT Identity with a per-partition bias
                # that plants the +/-BIG sentinel at the image boundary rows
                # (matmul writes exact 0.0 there), so the combines below
                # cover all 128 partitions with no boundary special case.
                bc = 0 if op == OP.min else 2
                nc.tensor.matmul(pu[:, 0], sup[:], src[:, 0, 3, :])
                nc.tensor.matmul(pu[:, 1], sup[:], src[:, 1, 3, :])
                nc.scalar.activation(out=t5[:, :, 0, :], in_=pu,
                                     func=AF.Identity,
                                     bias=bvec[:, bc:bc + 1])  # f32 -> bf16
                nc.tensor.matmul(pd[:, 0], sdn[:], src[:, 0, 0, :])
                nc.tensor.matmul(pd[:, 1], sdn[:], src[:, 1, 0, :])
                nc.scalar.activation(out=t5[:, :, 4, :], in_=pd,
                                     func=AF.Identity,
                                     bias=bvec[:, bc + 1:bc + 2])
                tt(t5[:, :, 1:4, :], src[:, :, 0:3, :],
                   src[:, :, 1:4, :], op)
                tt(dst[:, :, 0:4, :], t5[:, :, 0:4, :],
                   t5[:, :, 1:5, :], op)

            for ch in range(CHUNKS):
                x, other = (xa, xb) if ch % 2 == 0 else (xb, xa)
                # spread the load over the three available DMA queues
                nc.sync.dma_start(out=stage[:, 0, 0:2], in_=pred_v[ch, :, 0:2])
                nc.scalar.dma_start(out=stage[:, 0, 2:4],
                                    in_=pred_v[ch, :, 2:4])
                nc.gpsimd.dma_start(out=stage[:, 1, 0:2],
                                    in_=targ_v[ch, :, 0:2])
                nc.sync.dma_start(out=stage[:, 1, 2:4],
                                  in_=targ_v[ch, :, 2:4])
                nc.scalar.copy(out=x[:, 0], in_=stage[:, 0])  # f32 -> bf16
                nc.scalar.copy(out=x[:, 1], in_=stage[:, 1])

                for it in range(ITERS):
                    mh = other
                    hpool(mh, x, OP.min)      # mh = minc3(x)
                    vpool(m, mh, OP.min)      # m = minpool3(x)
                    if GPFUSE:
                        # c = x + m on GpSimd, overlapped with max-pool
                        nc.gpsimd.tensor_tensor(out=t, in0=x, in1=m,
                                                op=OP.add)
                    hpool(mh, m, OP.max)      # mh reused for maxc3(m)
                    vpool(Mh, mh, OP.max)     # Mh = M = maxpool3(m)
                    out_x = sk if it == ITERS - 1 else mh
                    if GPFUSE:
                        tt(out_x[:, :, :, :], t[:, :, :, :], Mh[:, :, :, :],
                           OP.subtract)       # x' = (x+m) - M
                    else:
                        tt(t[:, :, :, :], Mh[:, :, :, :], m[:, :, :, :],
                           OP.subtract)       # contour
                        tt(out_x[:, :, :, :], x[:, :, :, :], t[:, :, :, :],
                           OP.subtract)
                    if it < ITERS - 1:
                        x, other = mh, x

                # ncnt = 3x3 sum-pool of sk, all bf16
                tt(scr[:, :, :, 0:511], sk[:, :, :, 0:511],
                   sk[:, :, :, 1:512], OP.add)
                tt(shb[:, :, 3, 1:511], scr[:, :, 3, 0:510],
                   sk[:, :, 3, 2:512], OP.add)
                tt(shb[:, :, 0, 1:511], scr[:, :, 0, 0:510],
                   sk[:, :, 0, 2:512], OP.add)
                tt(shb[:, :, 1:3, 1:511], scr[:, :, 1:3, 0:510],
                   sk[:, :, 1:3, 2:512], OP.add)
                nc.scalar.copy(out=shb[:, :, :, 0:1], in_=scr[:, :, :, 0:1])
                nc.scalar.copy(out=shb[:, :, :, 511:512],
                               in_=scr[:, :, :, 510:511])
                # vertical sum via slot pairs + cross-partition shift rows
                # (dedicated qu/qd/pu2/pd2 so this phase never blocks the
                # next chunk's morphology shift pipeline)
                nc.tensor.matmul(pu2[:, 0], sup[:], shb[:, 0, 3, :])
                nc.tensor.matmul(pu2[:, 1], sup[:], shb[:, 1, 3, :])
                nc.scalar.copy(out=qu, in_=pu2)               # f32 -> bf16
                nc.tensor.matmul(pd2[:, 0], sdn[:], shb[:, 0, 0, :])
                nc.tensor.matmul(pd2[:, 1], sdn[:], shb[:, 1, 0, :])
                nc.scalar.copy(out=qd, in_=pd2)  # row127 = 0 (clipped sum)
                tt(scr[:, :, 1:4, :], shb[:, :, 0:3, :], shb[:, :, 1:4, :],
                   OP.add)
                tt(ncb[:, :, 1:3, :], scr[:, :, 1:3, :], shb[:, :, 2:4, :],
                   OP.add)
                tt(ncb[:, :, 0, :], scr[:, :, 1, :], qu[:], OP.add)
                tt(ncb[:, :, 3, :], scr[:, :, 3, :], qd[:], OP.add)
                # on = sk > 0.5 ; ep = (ncnt == 2)*on ; cr = (ncnt >= 4)*on
                # (tensor_scalar runs 4x on bf16; stt has no 2x uop)
                nc.vector.tensor_scalar(out=onb[:, :, :, :],
                                        in0=sk[:, :, :, :],
                                        scalar1=0.5, scalar2=None,
                                        op0=OP.is_gt)
                nc.vector.tensor_scalar(out=shb[:, :, :, :],
                                        in0=ncb[:, :, :, :],
                                        scalar1=2.0, scalar2=None,
                                        op0=OP.is_equal)
                tt(epb[:, :, :, :], shb[:, :, :, :], onb[:, :, :, :],
                   OP.mult)
                nc.vector.tensor_scalar(out=shb[:, :, :, :],
                                        in0=ncb[:, :, :, :],
                                        scalar1=4.0, scalar2=None,
                                        op0=OP.is_ge)
                tt(crb[:, :, :, :], shb[:, :, :, :], onb[:, :, :, :],
                   OP.mult)
                # squared-diff partial sums: diff on DVE (bf16 2x),
                # square + row-sum on ScalarE (Square + accum_out, f32)
                for k, buf in enumerate((sk, epb, crb)):
                    tt(scr[:, 0], buf[:, 0], buf[:, 1], OP.subtract)
                    nc.scalar.activation(
                        out=scr[:, 1], in_=scr[:, 0], func=AF.Square,
                        accum_out=pt[:, ch * 3 + k: ch * 3 + k + 1])

            nc.sync.dma_start(out=parts[:], in_=pt)

    _split_waits(nc, limit=1)
    return nc


def _run(pred_np, targ_np, trace=False):
    if "nc" not in _cache:
        _cache["nc"] = _build()
    nc = _cache["nc"]
    sup, sdn, bvec = _shift_mats()
    in_maps = []
    for c in range(NCORES):
        in_maps.append({
            "pred": np.ascontiguousarray(pred_np[c * CHUNKS:(c + 1) * CHUNKS]),
            "targ": np.ascontiguousarray(targ_np[c * CHUNKS:(c + 1) * CHUNKS]),
            "sup": sup, "sdn": sdn, "bvec": bvec,
        })
    return run_bass_kernel_spmd(nc, in_maps, core_ids=list(range(NCORES)),
                                trace=trace)


def kernel(pred, target):
    pred_np = np.asarray(pred, dtype=np.float32).reshape(32, H, W)
    targ_np = np.asarray(target, dtype=np.float32).reshape(32, H, W)
    res = _run(pred_np, targ_np)
    sums = np.zeros(3, dtype=np.float64)
    for r in res.results:
        p = r["partials"].astype(np.float64).reshape(P, CHUNKS, 3)
        sums += p.sum(axis=(0, 1))
    n = 32.0 * H * W
    loss = 0.6 * sums[0] / n + 0.2 * sums[1] / n + 0.2 * sums[2] / n
    return np.float32(loss)


# revision 21
# speedup vs baseline: 1.5189x; 1.0013x over previous
"""ConnectivityLoss Trainium2 Bass kernel.

Problem (hardcoded): pred/target (32, 1, 512, 512) f32.
  5 iterations of soft-skeletonize (3x3 min-pool -> 3x3 max-pool ->
  x = x - (M - m); both reference relus are provably no-ops), then 3x3
  sum-pool, endpoint/crossing masks, and a weighted MSE of the three
  pairs.

Sharding: pure data parallel over the batch dim; core i processes image
pairs 4i..4i+3 and returns per-partition partial sums of squared diffs;
the host sums and normalizes.

Per-core layout: partition p (128) owns image rows 4p..4p+3.
Free dims: (side 2, rowslot 4, col 512), fully contiguous.

Everything on-device runs in bf16 (numpy-validated loss error ~5.7e-3
vs the 2e-2 gate): morphology, the 3x3 sum-pool, masks and the squared
diffs.  bf16 keeps every DVE tensor_tensor in the 2x perf mode (f32 tt
runs 1x).  MSE squares+row-sums run on the Scalar engine (ACT Square
with accum_out), and the per-iteration update is fused to a single DVE
subtract by computing c = x + m on the otherwise idle GpSimd engine
during the max-pool phase (x' = x - (M - m) = c - M).

Cross-partition row shifts (rows 4p-1 / 4p+4) run on the idle
TensorEngine as shifted-identity bf16 matmuls into PSUM; ScalarE
evacuates PSUM to bf16 SBUF rows.  The hpool combine is split into
slot3 / slot0 / slots1:2 ops so the shift matmuls of the following
vpool start ~2us earlier, hiding the matmul+evac latency that
otherwise stalls the DVE ~750ns per vpool.  Compute-engine partition
windows must start at 0/32/64/96, so image boundaries are handled with
a [0:1] clipped-window overwrite (top) and matmul/consumer restricted
to [0:127) plus a [96:128) copy (bottom).
"""
import numpy as np
import ml_dtypes

import concourse.bass as bass
import concourse.tile as tile
from concourse import mybir
from concourse.bass_utils import run_bass_kernel_spmd

F32 = mybir.dt.float32
BF16 = mybir.dt.bfloat16
OP = mybir.AluOpType
AF = mybir.ActivationFunctionType

BIG = 1.0e30
P = 128
NCORES = 8
CHUNKS = 4
H = W = 512
ITERS = 5
GPFUSE = False  # GpSimd tt shares the DVE SBUF port: measured -48% on DVE

_cache = {}


def _split_waits(nc, limit=1):
    """This walrus build rejects instructions with more than ~1 embedded
    sync wait; hoist waits into standalone EventSemaphore instructions."""
    counter = 0
    for fn in nc.m.functions:
        for bb in fn.blocks:
            lst = list(bb.instructions)
            out = []
            changed = False
            for ins in lst:
                si = ins.sync_info
                waits = list(si.on_wait) if si is not None else []
                if len(waits) > limit:
                    changed = True
                    for w in waits:
                        counter += 1
                        es = mybir.InstEventSemaphore(
                            name=f"I-wsplit-{counter}", ins=[], outs=[],
                            sync_info=mybir.SyncInfo(on_wait=[w], on_update=[]),
                            bass_nofuse=True,
                        )
                        es.engine = ins.engine
                        out.append(es)
                    ins.sync_info = mybir.SyncInfo(
                        on_wait=[], on_update=list(si.on_update))
                out.append(ins)
            if changed:
                bb.instructions = out
    return counter


def _shift_mats():
    sup = np.zeros((P, P), np.float32)   # psum[p] = rhs[p-1]; col 0 zero
    sdn = np.zeros((P, P), np.float32)   # psum[p] = rhs[p+1]; col 127 zero
    for p in range(1, P):
        sup[p - 1, p] = 1.0
    for p in range(P - 1):
        sdn[p + 1, p] = 1.0
    bvec = np.zeros((P, 4), np.float32)
    bvec[0, 0] = BIG      # min, shift-up sentinel at row 0
    bvec[127, 1] = BIG    # min, shift-down sentinel at row 511
    bvec[0, 2] = -BIG     # max
    bvec[127, 3] = -BIG
    return (sup.astype(ml_dtypes.bfloat16), sdn.astype(ml_dtypes.bfloat16),
            bvec)


def _build():
    nc = bass.Bass()
    pred = nc.dram_tensor("pred", [CHUNKS, H, W], F32, kind="ExternalInput")
    targ = nc.dram_tensor("targ", [CHUNKS, H, W], F32, kind="ExternalInput")
    supd = nc.dram_tensor("sup", [P, P], BF16, kind="ExternalInput")
    sdnd = nc.dram_tensor("sdn", [P, P], BF16, kind="ExternalInput")
    # per-partition bias columns for the shift-row evacuations:
    # col0 +BIG@p0, col1 +BIG@p127 (min), col2 -BIG@p0, col3 -BIG@p127 (max)
    bvcd = nc.dram_tensor("bvec", [P, 4], F32, kind="ExternalInput")
    parts = nc.dram_tensor("partials", [P, CHUNKS * 3], F32,
                           kind="ExternalOutput")
    pred_v = pred.rearrange("n (p s) c -> n p s c", s=4)
    targ_v = targ.rearrange("n (p s) c -> n p s c", s=4)

    with tile.TileContext(nc) as tc:
        with tc.tile_pool(name="bufs", bufs=1) as pool, \
             tc.tile_pool(name="ps", bufs=1, space="PSUM") as pp:
            sh4 = [P, 2, 4, W]
            # bf16 morphology buffers
            xa = pool.tile(sh4, BF16)
            xb = pool.tile(sh4, BF16)
            t = pool.tile(sh4, BF16)
            m = pool.tile(sh4, BF16)
            Mh = pool.tile(sh4, BF16)
            sk = pool.tile(sh4, BF16)
            t5 = pool.tile([P, 2, 5, W], BF16)
            tmin = pool.tile([P, 2, 4, W + 1], BF16)   # +BIG pad cols 0,512
            tmax = pool.tile([P, 2, 4, W + 1], BF16)   # -BIG pad cols 0,512
            # load + bf16 post buffers
            stage = pool.tile(sh4, F32)
            scr = pool.tile(sh4, BF16)
            shb = pool.tile(sh4, BF16)
            ncb = pool.tile(sh4, BF16)
            onb = pool.tile(sh4, BF16)
            epb = pool.tile(sh4, BF16)
            crb = pool.tile(sh4, BF16)
            sup = pool.tile([P, P], BF16)
            sdn = pool.tile([P, P], BF16)
            bvec = pool.tile([P, 4], F32)
            pt = pool.tile([P, CHUNKS * 3], F32)
            # shift-row landing zones for the post-pool phase (decoupled
            # from t5/pu/pd so the sum-pool overlaps the next chunk's
            # morphology)
            qu = pool.tile([P, 2, W], BF16)
            qd = pool.tile([P, 2, W], BF16)
            pu = pp.tile([P, 2, W], F32)
            pd = pp.tile([P, 2, W], F32)
            pu2 = pp.tile([P, 2, W], F32)
            pd2 = pp.tile([P, 2, W], F32)

            nc.sync.dma_start(out=sup, in_=supd[:])
            nc.sync.dma_start(out=sdn, in_=sdnd[:])
            nc.sync.dma_start(out=bvec, in_=bvcd[:])
            # only the pad columns (0 and 512) need the +/-BIG sentinel
            nc.vector.memset(tmin[:, :, :, 0:1], BIG)
            nc.vector.memset(tmin[:, :, :, W:W + 1], BIG)
            nc.vector.memset(tmax[:, :, :, 0:1], -BIG)
            nc.vector.memset(tmax[:, :, :, W:W + 1], -BIG)

            def tt(out, a, b, op):
                nc.vector.tensor_tensor(out=out, in0=a, in1=b, op=op)

            def hpool(dst, src, op, by_side=False):
                # dst = 3-wide col pool of src (SAME, clipped). The pair
                # temp has static +/-BIG pad cols, so the second op covers
                # the edge columns too.  The combine is split so slots 3/0
                # land first: they feed the next vpool's shift matmuls.
                # by_side splits the pair op so side 0 (pred) can start
                # before side 1 (targ) finishes loading.
                tp = tmin if op == OP.min else tmax
                if by_side:
                    tt(tp[:, 0, :, 1:512], src[:, 0, :, 0:511],
                       src[:, 0, :, 1:512], op)
                    tt(tp[:, 1, :, 1:512], src[:, 1, :, 0:511],
                       src[:, 1, :, 1:512], op)
                else:
                    tt(tp[:, :, :, 1:512], src[:, :, :, 0:511],
                       src[:, :, :, 1:512], op)
                if True:
                    tt(dst[:, :, 3, 0:512], tp[:, :, 3, 0:512],
                       tp[:, :, 3, 1:513], op)
                    tt(dst[:, :, 0, 0:512], tp[:, :, 0, 0:512],
                       tp[:, :, 0, 1:513], op)
                    tt(dst[:, :, 1:3, 0:512], tp[:, :, 1:3, 0:512],
                       tp[:, :, 1:3, 1:513], op)
                else:
                    tt(dst[:, :, :, 0:512], tp[:, :, :, 0:512],
                       tp[:, :, :, 1:513], op)

            def vpool(dst, src, op):
                # dst = 3-wide row pool of src across partitions;
                # t5 = [shift-up, pair01, pair12, pair23, shift-dn].
                # Shift rows land via ACT Identity with a per-partition bias
                # that plants the +/-BIG sentinel at the image boundary rows
                # (matmul writes exact 0.0 there), so the combines below
                # cover all 128 partitions with no boundary special case.
                bc = 0 if op == OP.min else 2
                nc.tensor.matmul(pu[:, 0], sup[:], src[:, 0, 3, :])
                nc.tensor.matmul(pu[:, 1], sup[:], src[:, 1, 3, :])
                nc.scalar.activation(out=t5[:, :, 0, :], in_=pu,
                                     func=AF.Identity,
                                     bias=bvec[:, bc:bc + 1])  # f32 -> bf16
                nc.tensor.matmul(pd[:, 0], sdn[:], src[:, 0, 0, :])
                nc.tensor.matmul(pd[:, 1], sdn[:], src[:, 1, 0, :])
                nc.scalar.activation(out=t5[:, :, 4, :], in_=pd,
                                     func=AF.Identity,
                                     bias=bvec[:, bc + 1:bc + 2])
                tt(t5[:, :, 1:4, :], src[:, :, 0:3, :],
                   src[:, :, 1:4, :], op)
                tt(dst[:, :, 0:4, :], t5[:, :, 0:4, :],
                   t5[:, :, 1:5, :], op)

            for ch in range(CHUNKS):
                x, other = (xa, xb) if ch % 2 == 0 else (xb, xa)
                # pred on two queues (its convert gates the first DVE op),
                # targ on the third + queued-behind slots
                nc.sync.dma_start(out=stage[:, 0, 0:2], in_=pred_v[ch, :, 0:2])
                nc.scalar.dma_start(out=stage[:, 0, 2:4],
                                    in_=pred_v[ch, :, 2:4])
                nc.gpsimd.dma_start(out=stage[:, 1, 0:2],
                                    in_=targ_v[ch, :, 0:2])
                nc.sync.dma_start(out=stage[:, 1, 2:4],
                                  in_=targ_v[ch, :, 2:4])
                nc.scalar.copy(out=x[:, 0], in_=stage[:, 0])  # f32 -> bf16
                nc.scalar.copy(out=x[:, 1], in_=stage[:, 1])

                for it in range(ITERS):
                    mh = other
                    hpool(mh, x, OP.min, by_side=(it == 0))
                    vpool(m, mh, OP.min)      # m = minpool3(x)
                    if GPFUSE:
                        # c = x + m on GpSimd, overlapped with max-pool
                        nc.gpsimd.tensor_tensor(out=t, in0=x, in1=m,
                                                op=OP.add)
                    hpool(mh, m, OP.max)      # mh reused for maxc3(m)
                    vpool(Mh, mh, OP.max)     # Mh = M = maxpool3(m)
                    out_x = sk if it == ITERS - 1 else mh
                    if GPFUSE:
                        tt(out_x[:, :, :, :], t[:, :, :, :], Mh[:, :, :, :],
                           OP.subtract)       # x' = (x+m) - M
                    else:
                        tt(t[:, :, :, :], Mh[:, :, :, :], m[:, :, :, :],
                           OP.subtract)       # contour
                        tt(out_x[:, :, :, :], x[:, :, :, :], t[:, :, :, :],
                           OP.subtract)
                    if it < ITERS - 1:
                        x, other = mh, x

                # ncnt = 3x3 sum-pool of sk, all bf16
                tt(scr[:, :, :, 0:511], sk[:, :, :, 0:511],
                   sk[:, :, :, 1:512], OP.add)
                tt(shb[:, :, 3, 1:511], scr[:, :, 3, 0:510],
                   sk[:, :, 3, 2:512], OP.add)
                tt(shb[:, :, 0, 1:511], scr[:, :, 0, 0:510],
                   sk[:, :, 0, 2:512], OP.add)
                tt(shb[:, :, 1:3, 1:511], scr[:, :, 1:3, 0:510],
                   sk[:, :, 1:3, 2:512], OP.add)
                nc.scalar.copy(out=shb[:, :, :, 0:1], in_=scr[:, :, :, 0:1])
                nc.scalar.copy(out=shb[:, :, :, 511:512],
                               in_=scr[:, :, :, 510:511])
                # vertical sum via slot pairs + cross-partition shift rows
                # (dedicated qu/qd/pu2/pd2 so this phase never blocks the
                # next chunk's morphology shift pipeline)
                nc.tensor.matmul(pu2[:, 0], sup[:], shb[:, 0, 3, :])
                nc.tensor.matmul(pu2[:, 1], sup[:], shb[:, 1, 3, :])
                nc.scalar.copy(out=qu, in_=pu2)               # f32 -> bf16
                nc.tensor.matmul(pd2[:, 0], sdn[:], shb[:, 0, 0, :])
                nc.tensor.matmul(pd2[:, 1], sdn[:], shb[:, 1, 0, :])
                nc.scalar.copy(out=qd, in_=pd2)  # row127 = 0 (clipped sum)
                tt(scr[:, :, 1:4, :], shb[:, :, 0:3, :], shb[:, :, 1:4, :],
                   OP.add)
                tt(ncb[:, :, 1:3, :], scr[:, :, 1:3, :], shb[:, :, 2:4, :],
                   OP.add)
                tt(ncb[:, :, 0, :], scr[:, :, 1, :], qu[:], OP.add)
                tt(ncb[:, :, 3, :], scr[:, :, 3, :], qd[:], OP.add)
                # on = sk > 0.5 ; ep = (ncnt == 2)*on ; cr = (ncnt >= 4)*on
                # (tensor_scalar runs 4x on bf16; stt has no 2x uop)
                nc.vector.tensor_scalar(out=onb[:, :, :, :],
                                        in0=sk[:, :, :, :],
                                        scalar1=0.5, scalar2=None,
                                        op0=OP.is_gt)
                nc.vector.tensor_scalar(out=shb[:, :, :, :],
                                        in0=ncb[:, :, :, :],
                                        scalar1=2.0, scalar2=None,
                                        op0=OP.is_equal)
                tt(epb[:, :, :, :], shb[:, :, :, :], onb[:, :, :, :],
                   OP.mult)
                nc.vector.tensor_scalar(out=shb[:, :, :, :],
                                        in0=ncb[:, :, :, :],
                                        scalar1=4.0, scalar2=None,
                                        op0=OP.is_ge)
                tt(crb[:, :, :, :], shb[:, :, :, :], onb[:, :, :, :],
                   OP.mult)
                # squared-diff partial sums: diff on DVE (bf16 2x),
                # square + row-sum on ScalarE (Square + accum_out, f32)
                for k, buf in enumerate((sk, epb, crb)):
                    tt(scr[:, 0], buf[:, 0], buf[:, 1], OP.subtract)
                    nc.scalar.activation(
                        out=scr[:, 1], in_=scr[:, 0], func=AF.Square,
                        accum_out=pt[:, ch * 3 + k: ch * 3 + k + 1])
                # stream this chunk's partials out so the final DMA only
                # waits on the last chunk
                nc.sync.dma_start(out=parts[:, ch * 3: ch * 3 + 3],
                                  in_=pt[:, ch * 3: ch * 3 + 3])

    _split_waits(nc, limit=1)
    return nc


def _run(pred_np, targ_np, trace=False):
    if "nc" not in _cache:
        _cache["nc"] = _build()
    nc = _cache["nc"]
    sup, sdn, bvec = _shift_mats()
    in_maps = []
    for c in range(NCORES):
        in_maps.append({
            "pred": np.ascontiguousarray(pred_np[c * CHUNKS:(c + 1) * CHUNKS]),
            "targ": np.ascontiguousarray(targ_np[c * CHUNKS:(c + 1) * CHUNKS]),
            "sup": sup, "sdn": sdn, "bvec": bvec,
        })
    return run_bass_kernel_spmd(nc, in_maps, core_ids=list(range(NCORES)),
                                trace=trace)


def kernel(pred, target):
    pred_np = np.asarray(pred, dtype=np.float32).reshape(32, H, W)
    targ_np = np.asarray(target, dtype=np.float32).reshape(32, H, W)
    res = _run(pred_np, targ_np)
    sums = np.zeros(3, dtype=np.float64)
    for r in res.results:
        p = r["partials"].astype(np.float64).reshape(P, CHUNKS, 3)
        sums += p.sum(axis=(0, 1))
    n = 32.0 * H * W
    loss = 0.6 * sums[0] / n + 0.2 * sums[1] / n + 0.2 * sums[2] / n
    return np.float32(loss)


# revision 25
# speedup vs baseline: 1.5942x; 1.0496x over previous
"""ConnectivityLoss Trainium2 Bass kernel.

Problem (hardcoded): pred/target (32, 1, 512, 512) f32.
  5 iterations of soft-skeletonize (3x3 min-pool -> 3x3 max-pool ->
  x = x - (M - m); both reference relus are provably no-ops), then 3x3
  sum-pool, endpoint/crossing masks, and a weighted MSE of the three
  pairs.

Sharding: pure data parallel over the batch dim; core i processes image
pairs 4i..4i+3 and returns per-partition partial sums of squared diffs;
the host sums and normalizes.

Per-core layout: partition p (128) owns image rows 4p..4p+3.
Free dims: (side 2, rowslot 4, col 512), fully contiguous.

Everything on-device runs in bf16 (numpy-validated loss error ~5.7e-3
vs the 2e-2 gate).  bf16 keeps every DVE tensor_tensor in the 2x perf
mode (f32 tt runs 1x).  MSE squares+row-sums run on the Scalar engine
(ACT Square with accum_out).

The four chunks are processed as two interleaved streams (chunk pairs
(0,1) and (2,3)): per iteration the emission alternates stream A /
stream B so each stream's TensorE shift-matmul + ACT evacuation
latencies hide under the other stream's DVE block, and each stream's
post-pool overlaps the other's morphology.  Each stream owns its
x/m/M/sk/t5/stage buffers; the hpool pair scratch (tmin/tmax) is
shared (adjacent-op WAR, zero cost).  Post-pool tensors alias the
stream's dead morphology buffers.

Cross-partition row shifts (rows 4p-1 / 4p+4) run on the idle
TensorEngine as shifted-identity bf16 matmuls into PSUM; ScalarE
evacuates PSUM to bf16 SBUF rows via an Identity activation whose
per-partition bias plants +/-BIG sentinels at the image boundary rows
(the shift matrices write exact 0.0 there), so pool combines cover all
128 partitions with no boundary special case.  The hpool combine is
split into slot3 / slot0 / slots1:2 ops so the following vpool's shift
matmuls start ~2us early.
"""
import numpy as np
import ml_dtypes

import concourse.bass as bass
import concourse.tile as tile
from concourse import mybir
from concourse.bass_utils import run_bass_kernel_spmd

F32 = mybir.dt.float32
BF16 = mybir.dt.bfloat16
OP = mybir.AluOpType
AF = mybir.ActivationFunctionType

BIG = 1.0e30
P = 128
NCORES = 8
CHUNKS = 4
H = W = 512
ITERS = 5

_cache = {}


def _split_waits(nc, limit=1):
    """This walrus build rejects instructions with more than ~1 embedded
    sync wait; hoist waits into standalone EventSemaphore instructions."""
    counter = 0
    for fn in nc.m.functions:
        for bb in fn.blocks:
            lst = list(bb.instructions)
            out = []
            changed = False
            for ins in lst:
                si = ins.sync_info
                waits = list(si.on_wait) if si is not None else []
                if len(waits) > limit:
                    changed = True
                    for w in waits:
                        counter += 1
                        es = mybir.InstEventSemaphore(
                            name=f"I-wsplit-{counter}", ins=[], outs=[],
                            sync_info=mybir.SyncInfo(on_wait=[w], on_update=[]),
                            bass_nofuse=True,
                        )
                        es.engine = ins.engine
                        out.append(es)
                    ins.sync_info = mybir.SyncInfo(
                        on_wait=[], on_update=list(si.on_update))
                out.append(ins)
            if changed:
                bb.instructions = out
    return counter


def _shift_mats():
    sup = np.zeros((P, P), np.float32)   # psum[p] = rhs[p-1]; col 0 zero
    sdn = np.zeros((P, P), np.float32)   # psum[p] = rhs[p+1]; col 127 zero
    for p in range(1, P):
        sup[p - 1, p] = 1.0
    for p in range(P - 1):
        sdn[p + 1, p] = 1.0
    bvec = np.zeros((P, 4), np.float32)
    bvec[0, 0] = BIG      # min, shift-up sentinel at row 0
    bvec[127, 1] = BIG    # min, shift-down sentinel at row 511
    bvec[0, 2] = -BIG     # max
    bvec[127, 3] = -BIG
    return (sup.astype(ml_dtypes.bfloat16), sdn.astype(ml_dtypes.bfloat16),
            bvec)


def _build():
    nc = bass.Bass()
    pred = nc.dram_tensor("pred", [CHUNKS, H, W], F32, kind="ExternalInput")
    targ = nc.dram_tensor("targ", [CHUNKS, H, W], F32, kind="ExternalInput")
    supd = nc.dram_tensor("sup", [P, P], BF16, kind="ExternalInput")
    sdnd = nc.dram_tensor("sdn", [P, P], BF16, kind="ExternalInput")
    bvcd = nc.dram_tensor("bvec", [P, 4], F32, kind="ExternalInput")
    parts = nc.dram_tensor("partials", [P, CHUNKS * 3], F32,
                           kind="ExternalOutput")
    pred_v = pred.rearrange("n (p s) c -> n p s c", s=4)
    targ_v = targ.rearrange("n (p s) c -> n p s c", s=4)

    with tile.TileContext(nc) as tc:
        with tc.tile_pool(name="bufs", bufs=1) as pool, \
             tc.tile_pool(name="ps", bufs=1, space="PSUM") as pp:
            sh4 = [P, 2, 4, W]
            NS = 2  # streams

            def per_stream(nm, shape, dt):
                return [pool.tile(shape, dt, name=f"{nm}{i}")
                        for i in range(NS)]

            xa = per_stream("xa", sh4, BF16)
            xb = per_stream("xb", sh4, BF16)
            t = per_stream("t", sh4, BF16)      # contour scratch
            m = per_stream("m", sh4, BF16)
            Mh = per_stream("Mh", sh4, BF16)
            sk = per_stream("sk", sh4, BF16)
            t5 = per_stream("t5", [P, 2, 5, W], BF16)
            stage = per_stream("stage", sh4, F32)
            qu = per_stream("qu", [P, 2, W], BF16)
            qd = per_stream("qd", [P, 2, W], BF16)
            # shared hpool pair scratch: +/-BIG pad cols 0,512
            tmin = pool.tile([P, 2, 4, W + 1], BF16)
            tmax = pool.tile([P, 2, 4, W + 1], BF16)
            sup = pool.tile([P, P], BF16)
            sdn = pool.tile([P, P], BF16)
            bvec = pool.tile([P, 4], F32)
            pt = pool.tile([P, CHUNKS * 3], F32)
            pu = [pp.tile([P, 2, W], F32, name=f"pu{i}") for i in range(NS)]
            pd = [pp.tile([P, 2, W], F32, name=f"pd{i}") for i in range(NS)]

            nc.sync.dma_start(out=sup, in_=supd[:])
            nc.sync.dma_start(out=sdn, in_=sdnd[:])
            nc.sync.dma_start(out=bvec, in_=bvcd[:])
            nc.vector.memset(tmin[:, :, :, 0:1], BIG)
            nc.vector.memset(tmin[:, :, :, W:W + 1], BIG)
            nc.vector.memset(tmax[:, :, :, 0:1], -BIG)
            nc.vector.memset(tmax[:, :, :, W:W + 1], -BIG)

            def tt(out, a, b, op):
                nc.vector.tensor_tensor(out=out, in0=a, in1=b, op=op)

            def hpool(dst, src, op, by_side=False):
                # dst = 3-wide col pool of src (SAME, clipped). The pair
                # temp has static +/-BIG pad cols, so the second op covers
                # the edge columns too.  The combine is split so slots 3/0
                # land first: they feed the next vpool's shift matmuls.
                # by_side splits the pair op so side 0 (pred) can start
                # before side 1 (targ) finishes loading.
                tp = tmin if op == OP.min else tmax
                if by_side:
                    tt(tp[:, 0, :, 1:512], src[:, 0, :, 0:511],
                       src[:, 0, :, 1:512], op)
                    tt(tp[:, 1, :, 1:512], src[:, 1, :, 0:511],
                       src[:, 1, :, 1:512], op)
                else:
                    tt(tp[:, :, :, 1:512], src[:, :, :, 0:511],
                       src[:, :, :, 1:512], op)
                tt(dst[:, :, 3, 0:512], tp[:, :, 3, 0:512],
                   tp[:, :, 3, 1:513], op)
                tt(dst[:, :, 0, 0:512], tp[:, :, 0, 0:512],
                   tp[:, :, 0, 1:513], op)
                tt(dst[:, :, 1:3, 0:512], tp[:, :, 1:3, 0:512],
                   tp[:, :, 1:3, 1:513], op)

            def vpool(s, dst, src, op):
                # dst = 3-wide row pool of src across partitions;
                # t5 = [shift-up, pair01, pair12, pair23, shift-dn].
                bc = 0 if op == OP.min else 2
                t5s, pus, pds = t5[s], pu[s], pd[s]
                nc.tensor.matmul(pus[:, 0], sup[:], src[:, 0, 3, :])
                nc.tensor.matmul(pus[:, 1], sup[:], src[:, 1, 3, :])
                nc.scalar.activation(out=t5s[:, :, 0, :], in_=pus,
                                     func=AF.Identity,
                                     bias=bvec[:, bc:bc + 1])  # f32 -> bf16
                nc.tensor.matmul(pds[:, 0], sdn[:], src[:, 0, 0, :])
                nc.tensor.matmul(pds[:, 1], sdn[:], src[:, 1, 0, :])
                nc.scalar.activation(out=t5s[:, :, 4, :], in_=pds,
                                     func=AF.Identity,
                                     bias=bvec[:, bc + 1:bc + 2])
                tt(t5s[:, :, 1:4, :], src[:, :, 0:3, :],
                   src[:, :, 1:4, :], op)
                tt(dst[:, :, 0:4, :], t5s[:, :, 0:4, :],
                   t5s[:, :, 1:5, :], op)

            # stream state: (cur_x, other)
            state = [None, None]

            def emit_load(s, ch):
                st = stage[s]
                nc.sync.dma_start(out=st[:, 0, 0:2], in_=pred_v[ch, :, 0:2])
                nc.scalar.dma_start(out=st[:, 0, 2:4],
                                    in_=pred_v[ch, :, 2:4])
                nc.gpsimd.dma_start(out=st[:, 1, 0:2],
                                    in_=targ_v[ch, :, 0:2])
                nc.sync.dma_start(out=st[:, 1, 2:4],
                                  in_=targ_v[ch, :, 2:4])
                x = xa[s]
                nc.scalar.copy(out=x[:, 0], in_=st[:, 0])  # f32 -> bf16
                nc.scalar.copy(out=x[:, 1], in_=st[:, 1])
                state[s] = (x, xb[s])

            def emit_iter(s, it):
                x, other = state[s]
                mh = other
                hpool(mh, x, OP.min, by_side=(it == 0))
                vpool(s, m[s], mh, OP.min)
                hpool(mh, m[s], OP.max)
                vpool(s, Mh[s], mh, OP.max)
                tt(t[s][:, :, :, :], Mh[s][:, :, :, :], m[s][:, :, :, :],
                   OP.subtract)          # contour
                out_x = sk[s] if it == ITERS - 1 else mh
                tt(out_x[:, :, :, :], x[:, :, :, :], t[s][:, :, :, :],
                   OP.subtract)
                if it < ITERS - 1:
                    state[s] = (mh, x)

            def emit_post(s, ch):
                # post tensors alias this stream's dead morphology buffers
                sks = sk[s]
                scr, shb, ncb, onb = m[s], Mh[s], state[s][1], t[s]
                # ncnt = 3x3 sum-pool of sk, all bf16
                tt(scr[:, :, :, 0:511], sks[:, :, :, 0:511],
                   sks[:, :, :, 1:512], OP.add)
                tt(shb[:, :, 3, 1:511], scr[:, :, 3, 0:510],
                   sks[:, :, 3, 2:512], OP.add)
                tt(shb[:, :, 0, 1:511], scr[:, :, 0, 0:510],
                   sks[:, :, 0, 2:512], OP.add)
                tt(shb[:, :, 1:3, 1:511], scr[:, :, 1:3, 0:510],
                   sks[:, :, 1:3, 2:512], OP.add)
                nc.scalar.copy(out=shb[:, :, :, 0:1], in_=scr[:, :, :, 0:1])
                nc.scalar.copy(out=shb[:, :, :, 511:512],
                               in_=scr[:, :, :, 510:511])
                # vertical sum via slot pairs + cross-partition shift rows
                nc.tensor.matmul(pu[s][:, 0], sup[:], shb[:, 0, 3, :])
                nc.tensor.matmul(pu[s][:, 1], sup[:], shb[:, 1, 3, :])
                nc.scalar.copy(out=qu[s], in_=pu[s])          # f32 -> bf16
                nc.tensor.matmul(pd[s][:, 0], sdn[:], shb[:, 0, 0, :])
                nc.tensor.matmul(pd[s][:, 1], sdn[:], shb[:, 1, 0, :])
                nc.scalar.copy(out=qd[s], in_=pd[s])  # row127 = 0 (clipped)
                tt(scr[:, :, 1:4, :], shb[:, :, 0:3, :], shb[:, :, 1:4, :],
                   OP.add)
                tt(ncb[:, :, 1:3, :], scr[:, :, 1:3, :], shb[:, :, 2:4, :],
                   OP.add)
                tt(ncb[:, :, 0, :], scr[:, :, 1, :], qu[s][:], OP.add)
                tt(ncb[:, :, 3, :], scr[:, :, 3, :], qd[s][:], OP.add)
                # on = sk > 0.5 ; ep = (ncnt == 2)*on ; cr = (ncnt >= 4)*on
                # (tensor_scalar runs 4x on bf16; masks multiply in place)
                nc.vector.tensor_scalar(out=onb[:, :, :, :],
                                        in0=sks[:, :, :, :],
                                        scalar1=0.5, scalar2=None,
                                        op0=OP.is_gt)
                # squared-diff partial sums: diff on DVE (bf16 2x),
                # square + row-sum on ScalarE (Square + accum_out, f32)
                tt(scr[:, 0], sks[:, 0], sks[:, 1], OP.subtract)
                nc.scalar.activation(
                    out=scr[:, 1], in_=scr[:, 0], func=AF.Square,
                    accum_out=pt[:, ch * 3: ch * 3 + 1])
                for k, op0 in ((1, OP.is_equal), (2, OP.is_ge)):
                    nc.vector.tensor_scalar(out=shb[:, :, :, :],
                                            in0=ncb[:, :, :, :],
                                            scalar1=2.0 if k == 1 else 4.0,
                                            scalar2=None, op0=op0)
                    tt(shb[:, :, :, :], shb[:, :, :, :], onb[:, :, :, :],
                       OP.mult)
                    tt(scr[:, 0], shb[:, 0], shb[:, 1], OP.subtract)
                    nc.scalar.activation(
                        out=scr[:, 1], in_=scr[:, 0], func=AF.Square,
                        accum_out=pt[:, ch * 3 + k: ch * 3 + k + 1])
                # stream this chunk's partials out
                nc.sync.dma_start(out=parts[:, ch * 3: ch * 3 + 3],
                                  in_=pt[:, ch * 3: ch * 3 + 3])

            for pair in range(CHUNKS // 2):
                chA, chB = 2 * pair, 2 * pair + 1
                emit_load(0, chA)
                emit_load(1, chB)
                for it in range(ITERS):
                    emit_iter(0, it)
                    emit_iter(1, it)
                emit_post(0, chA)
                emit_post(1, chB)

    _split_waits(nc, limit=1)
    return nc


def _run(pred_np, targ_np, trace=False):
    if "nc" not in _cache:
        _cache["nc"] = _build()
    nc = _cache["nc"]
    sup, sdn, bvec = _shift_mats()
    in_maps = []
    for c in range(NCORES):
        in_maps.append({
            "pred": np.ascontiguousarray(pred_np[c * CHUNKS:(c + 1) * CHUNKS]),
            "targ": np.ascontiguousarray(targ_np[c * CHUNKS:(c + 1) * CHUNKS]),
            "sup": sup, "sdn": sdn, "bvec": bvec,
        })
    return run_bass_kernel_spmd(nc, in_maps, core_ids=list(range(NCORES)),
                                trace=trace)


def kernel(pred, target):
    pred_np = np.asarray(pred, dtype=np.float32).reshape(32, H, W)
    targ_np = np.asarray(target, dtype=np.float32).reshape(32, H, W)
    res = _run(pred_np, targ_np)
    sums = np.zeros(3, dtype=np.float64)
    for r in res.results:
        p = r["partials"].astype(np.float64).reshape(P, CHUNKS, 3)
        sums += p.sum(axis=(0, 1))
    n = 32.0 * H * W
    loss = 0.6 * sums[0] / n + 0.2 * sums[1] / n + 0.2 * sums[2] / n
    return np.float32(loss)
